# revision 8
# baseline (speedup 1.0000x reference)
"""Trainium2 Bass kernel for nn_CustomTransformer_58445914964311 (v3).

12-layer MoE transformer (768 embd, 8 heads, 8 experts top-2, B=8 x T=64
tokens), distributed over 8 NeuronCores:
  - attention sharded by head (core c computes head c for all tokens);
    per-head outputs o are AllGathered in fp16 (~7us) and the full output
    projection is computed redundantly on every core, replacing the
    baseline's AllReduce of projection partials.
  - MoE sharded by expert (core c computes expert c densely over all 512
    tokens -- the top-2 routing is too imbalanced to exploit, counts per
    (layer, expert) range 3..426 of 512). Partials (including the b2
    expert bias, folded in pre-AllReduce) are AllReduced in fp16 (~20us
    per 256-token half).
  - the residual stream, layernorms, softmaxes and gate logits stay
    fp32: gate top-2 margins go down to 2.6e-6 and even 1e-5-level
    stream noise flips expert choices (measured: fp16 LN sum-of-squares
    or fp16 b2 bias each cost ~0.25 abs rel-err). All heavy matmuls
    (qkv, proj, w1, w2, lm head) run in fp16 at 1 cyc/row.

Schedule: two 256-token halves (batch rows 0-3 / 4-7; causal attention
never crosses rows). Each half's FFN weight-stream is interleaved at
m-tile granularity with the other half's projection/LN/gate (or ln2)
chunks so the PE queue stays backlogged while vector/scalar chains
drain; AllReduces ride under the other half's FFN. Weight tensors are
laid out host-side so every DMA is contiguous per partition line
(256-byte strided tile loads starved the PE at m-tile boundaries).

Perf history on this fabric: fp32 baseline 5.47ms -> fp16 + o-AllGather
3.14ms -> contiguous weight DMA 2.82ms -> b2-fold + batched LN sums
2.74ms -> batched attention/gate + pool rebalance 2.60ms -> deeper
w1/w2 prefetch + earlier next-layer weight loads + denser w2-phase
filler slots 2.54ms. Run-to-run variance on this fabric is ~5-10%.

Self-contained: hardcodes all shapes; host side only reshapes/casts.
"""

import numpy as np

import concourse.bass as bass
import concourse.mybir as mybir
import concourse.tile as tile
from concourse.bass_utils import run_bass_kernel_spmd

import os
import sys

# ---------------------------------------------------------------------------
# Compatibility patches (inlined): the walrus build here rejects instructions
# carrying more than one semaphore wait ("Too many sync wait commands").
# ---------------------------------------------------------------------------
import orjson as _orjson
from concourse.vector_clock import ScopedClock as _ScopedClock

_COMPAT_DONE = False


def _patched_drain_and_barrier(self, tick_clock, wait_clock):
    nc = self.nc
    collector = nc.sync.nop()
    wait_clock.add_sem_waits(
        collector.ins, _ScopedClock({None: tick_clock.global_clock})
    )
    si = collector.ins.sync_info
    waits = list(si.on_wait or []) if si is not None else []
    if len(waits) > 1:
        si.on_wait = waits[:1]
        for w in waits[1:]:
            extra = nc.sync.nop()
            esi = extra.ins.sync_info
            if esi is None:
                extra.ins.sync_info = mybir.SyncInfo(on_wait=[w], on_update=[])
            else:
                esi.on_wait = [w]
    nc.sync.drain()
    nc.all_engine_barrier()
    popped = nc._tile_sem_poison_stack.pop()
    assert popped is self._sem_poison
    nc.clear_and_free_semaphores(list(self.sems.allocated().values()))
    nc.all_engine_barrier()


def _split_multi_waits(mod, max_waits=1):
    ctr = 0
    for fn in mod.get("functions", []):
        for blk in fn.get("blocks", []):
            insts = blk.get("instructions", [])
            if not any(
                len((i.get("sync_info") or {}).get("on_wait") or []) > max_waits
                for i in insts
            ):
                continue
            new_insts = []
            for inst in insts:
                si = inst.get("sync_info")
                waits = (si.get("on_wait") or []) if si else []
                if len(waits) > max_waits:
                    for w in waits[max_waits:]:
                        ctr += 1
                        new_insts.append({
                            "debug": inst.get("debug", 0),
                            "engine": inst["engine"],
                            "ins": [], "outs": [],
                            "name": f"{inst['name']}-wsp{ctr}",
                            "opcode": "EventSemaphore",
                            "sync_info": {"on_update": [], "on_wait": [w]},
                        })
                    si["on_wait"] = waits[:max_waits]
                new_insts.append(inst)
            blk["instructions"] = new_insts
    return mod


_orig_to_json_bytes = bass.Bass.to_json_bytes


def _patched_to_json_bytes(self):
    return _orjson.dumps(_split_multi_waits(_orjson.loads(_orig_to_json_bytes(self))))


def _install_ntff_hook_shim():
    import types
    if "antenv.axon_hooks" in sys.modules:
        return
    try:
        import antenv  # noqa: F401
    except ImportError:
        return
    mod = types.ModuleType("antenv.axon_hooks")
    _state = {"hook": None}
    mod.set_axon_ntff_profile_hook = lambda hook: _state.__setitem__("hook", hook)
    mod.get_axon_ntff_profile_hook = lambda: _state["hook"]
    sys.modules["antenv.axon_hooks"] = mod
    sys.modules["antenv"].axon_hooks = mod
    try:
        from trn_agent_boot.trn_boot import _ntff_profile_via_ctypes
        hook = _ntff_profile_via_ctypes("/opt/axon/libaxon_pjrt.so")
        if hook is not None:
            mod.set_axon_ntff_profile_hook(hook)
    except Exception:
        pass


def _install_compat():
    global _COMPAT_DONE
    if _COMPAT_DONE:
        return
    tile.TileContext._drain_and_barrier = _patched_drain_and_barrier
    bass.Bass.to_json_bytes = _patched_to_json_bytes
    _install_ntff_hook_shim()
    _COMPAT_DONE = True


_install_compat()

F32 = mybir.dt.float32
F16 = mybir.dt.float16
I32 = mybir.dt.int32
AF = mybir.ActivationFunctionType
ALU = mybir.AluOpType
AX = mybir.AxisListType

N_CORES = 8
L = 12
D = 768
H = 96          # head dim
NH = 8
E = 8           # experts
DFF = 3072
B, T = 8, 64
N = B * T       # 512 tokens
NHLF = N // 2   # 256 tokens per pipeline half
V = 99
KT = D // 128   # 6 feature tiles
MT = DFF // 128  # 24 dff tiles
EPS = 1e-5
SCALE = H ** -0.5

_CACHED = {}


def build():
    nc = bass.Bass(num_devices=N_CORES)

    # ---- inputs (per-core data, same names) ----
    d_idx = nc.dram_tensor("idx", [1, N], I32, kind="ExternalInput")
    d_iota = nc.dram_tensor("iota99", [V, 1], F32, kind="ExternalInput")
    d_ident = nc.dram_tensor("ident128", [128, 128], F32, kind="ExternalInput")
    d_mask = nc.dram_tensor("maskb", [64, 64], F32, kind="ExternalInput")
    d_ones_col = nc.dram_tensor("ones_col", [128, 1], F32, kind="ExternalInput")
    d_ones_row = nc.dram_tensor("ones_row", [1, 128], F32, kind="ExternalInput")
    d_tok = nc.dram_tensor("tok_emb", [V, D], F32, kind="ExternalInput")
    d_posT = nc.dram_tensor("posT", [D, N], F32, kind="ExternalInput")
    d_wqT = nc.dram_tensor("wqT", [L, 128, KT, H], F16, kind="ExternalInput")
    d_wkT = nc.dram_tensor("wkT", [L, 128, KT, H], F16, kind="ExternalInput")
    d_wvT = nc.dram_tensor("wvT", [L, 128, KT, H], F16, kind="ExternalInput")
    d_wpT = nc.dram_tensor("wpT", [L, 128, KT, D], F16, kind="ExternalInput")
    d_bproj = nc.dram_tensor("bproj", [L, 128, KT], F32, kind="ExternalInput")
    d_gwT = nc.dram_tensor("gwT", [L, 128, KT, E], F32, kind="ExternalInput")
    d_gb = nc.dram_tensor("gb", [L, 1, E], F32, kind="ExternalInput")
    d_b1 = nc.dram_tensor("b1", [L, 128, MT], F32, kind="ExternalInput")
    d_b2c = nc.dram_tensor("b2own", [L, 128, KT], F32, kind="ExternalInput")
    d_combsel = nc.dram_tensor("combsel", [E, 1], F32, kind="ExternalInput")
    d_ln1w = nc.dram_tensor("ln1w", [L, 128, KT], F32, kind="ExternalInput")
    d_ln1b = nc.dram_tensor("ln1b", [L, 128, KT], F32, kind="ExternalInput")
    d_ln2w = nc.dram_tensor("ln2w", [L, 128, KT], F32, kind="ExternalInput")
    d_ln2b = nc.dram_tensor("ln2b", [L, 128, KT], F32, kind="ExternalInput")
    d_lnfw = nc.dram_tensor("lnfw", [128, KT], F32, kind="ExternalInput")
    d_lnfb = nc.dram_tensor("lnfb", [128, KT], F32, kind="ExternalInput")
    d_lmT = nc.dram_tensor("lmT", [KT, 128, V], F16, kind="ExternalInput")
    d_w1h = nc.dram_tensor("w1h", [L, MT, 128, KT, 128], F16, kind="ExternalInput")
    d_w2h = nc.dram_tensor("w2h", [L, KT, 128, MT, 128], F16, kind="ExternalInput")
    d_dar = nc.dram_tensor("dar", [1, 8], F32, kind="ExternalInput")
    d_epsb = nc.dram_tensor("epsb", [1, 1], F32, kind="ExternalInput")
    d_lmb = nc.dram_tensor("lmb", [V, 1], F32, kind="ExternalInput")
    d_out = nc.dram_tensor("logitsT", [V, N], F32, kind="ExternalOutput")

    with tile.TileContext(nc) as tc:
        with (
            tc.tile_pool(name="const", bufs=1) as cpool,
            tc.tile_pool(name="x", bufs=1) as xpool,
            tc.tile_pool(name="attw", bufs=2) as awpool,
            tc.tile_pool(name="w1", bufs=10) as w1pool,
            tc.tile_pool(name="w2", bufs=5) as w2pool,
            tc.tile_pool(name="h", bufs=1) as hpool,
            tc.tile_pool(name="work", bufs=2) as wk,
            tc.tile_pool(name="small", bufs=3) as sm,
            tc.tile_pool(name="ps_acc", bufs=3, space="PSUM") as ps_acc,
            tc.tile_pool(name="ps_bc", bufs=1, space="PSUM") as ps_bc,
            tc.tile_pool(name="ps_ln", bufs=2, space="PSUM") as ps_ln,
            tc.tile_pool(name="ps_att", bufs=2, space="PSUM") as ps_att,
            tc.tile_pool(name="dram", bufs=1, space="DRAM") as dpool,
        ):
            # ---- constants resident ----
            ident = cpool.tile([128, 128], F32, name="ident")
            nc.sync.dma_start(ident[:], d_ident[:])
            maskb = cpool.tile([64, 64], F32, name="maskb")
            nc.sync.dma_start(maskb[:], d_mask[:])
            mask4 = cpool.tile([64, NHLF], F32, name="mask4")
            for _b in range(4):
                nc.vector.tensor_copy(mask4[:, _b * 64:(_b + 1) * 64],
                                      maskb[:])
            iota99 = cpool.tile([V, 1], F32, name="iota99")
            nc.sync.dma_start(iota99[:], d_iota[:])
            ones_col = cpool.tile([128, 1], F32, name="ones_col")
            nc.sync.dma_start(ones_col[:], d_ones_col[:])
            ones_row = cpool.tile([1, 128], F32, name="ones_row")
            nc.sync.dma_start(ones_row[:], d_ones_row[:])
            combsel = cpool.tile([E, 1], F32, name="combsel")
            nc.sync.dma_start(combsel[:], d_combsel[:])
            epsb = cpool.tile([1, 1], F32, name="epsb")
            nc.sync.dma_start(epsb[:], d_epsb[:])
            tok = cpool.tile([V, D], F32, name="tok")
            nc.sync.dma_start(tok[:], d_tok[:])
            lmT = cpool.tile([128, KT * V], F16, name="lmT")
            for k in range(KT):
                nc.sync.dma_start(lmT[:, k * V:(k + 1) * V], d_lmT[k])
            lmb = cpool.tile([V, 1], F32, name="lmb")
            nc.sync.dma_start(lmb[:], d_lmb[:])
            lnfw = cpool.tile([128, KT], F32, name="lnfw")
            nc.sync.dma_start(lnfw[:], d_lnfw[:])
            lnfb = cpool.tile([128, KT], F32, name="lnfb")
            nc.sync.dma_start(lnfb[:], d_lnfb[:])

            ones_col16 = cpool.tile([128, 1], F16, name="ones_col16")
            nc.scalar.activation(ones_col16[:], ones_col[:], AF.Identity)

            dar_i = dpool.tile([1, 8], F32, name="dar_i")
            nc.sync.dma_start(dar_i[:], d_dar[:])
            dar_o = dpool.tile([1, 8], F32, name="dar_o", addr_space="Shared")
            nc.gpsimd.collective_compute(
                "AllReduce", ALU.add,
                replica_groups=[list(range(N_CORES))],
                ins=[dar_i[:]], outs=[dar_o[:]])

            # AG bounce tensors (per-head o, fp16) and AR tensors (ffn, fp16)
            ag_o_i = [[dpool.tile([H, NHLF], F16, name=f"agi{l}_{h}")
                       for h in range(2)] for l in range(L)]
            ag_o_o = [[dpool.tile([H * N_CORES, NHLF], F16, name=f"ago{l}_{h}",
                                  addr_space="Shared")
                       for h in range(2)] for l in range(L)]
            ar_ffn_i = [[dpool.tile([D, NHLF], F16, name=f"afi{l}_{h}")
                         for h in range(2)] for l in range(L)]
            ar_ffn_o = [[dpool.tile([D, NHLF], F16, name=f"afo{l}_{h}",
                                    addr_space="Shared")
                         for h in range(2)] for l in range(L)]

            # ---- x state: 6 tiles [128, N] fp32 + fp16 shadow ----
            x_sb = xpool.tile([128, KT * N], F32, name="x_sb")
            x16 = xpool.tile([128, KT * N], F16, name="x16")

            def xs(k, hs=slice(0, N)):
                return x_sb[:, k * N + hs.start:k * N + hs.stop]

            def xs16(k, hs=slice(0, N)):
                return x16[:, k * N + hs.start:k * N + hs.stop]

            def hslice(h):
                return slice(h * NHLF, (h + 1) * NHLF)

            # ---- embedding (full batch) ----
            idx_i = sm.tile([1, N], I32, name="idx_i", bufs=1)
            nc.sync.dma_start(idx_i[:], d_idx[:])
            idx_f = sm.tile([1, N], F32, name="idx_f", bufs=1)
            nc.vector.tensor_copy(idx_f[:], idx_i[:])
            onehot = wk.tile([V, N], F32, name="onehot", bufs=1)
            for h in range(2):
                hs = hslice(h)
                idxbc = ps_att.tile([V, NHLF], F32, tag="a", name="idxbc")
                nc.tensor.matmul(idxbc[:], ones_row[:, :V], idx_f[:, hs],
                                 start=True, stop=True)
                nc.vector.tensor_scalar(onehot[:, hs], idxbc[:], iota99[:],
                                        None, op0=ALU.is_equal)
            for k in range(KT):
                posk = sm.tile([128, N], F32, tag="posk", bufs=2)
                nc.sync.dma_start(posk[:], d_posT[k * 128:(k + 1) * 128, :])
                for h in range(2):
                    hs = hslice(h)
                    e_ps = ps_acc.tile([128, NHLF], F32, tag="acc")
                    nc.tensor.matmul(e_ps[:], tok[:, k * 128:(k + 1) * 128],
                                     onehot[:, hs], start=True, stop=True)
                    nc.vector.tensor_add(xs(k, hs), e_ps[:], posk[:, hs])
                    nc.scalar.activation(xs16(k, hs), xs(k, hs), AF.Identity)

            def layernorm_emit(q, h, get_t, w_ap, b_ap, make16, uniq,
                               tmp=None):
                """Append LN chunks to q: t chunks, batched sums, finisher,
                6 norm chunks. If tmp given, t is already materialized."""
                hs = hslice(h)
                if tmp is None:
                    tmp = wk.tile([128, KT * NHLF], F32, name=f"ln_t{uniq}",
                                  bufs=1)

                    def mk_t(k):
                        def go():
                            get_t(k, tmp[:, k * NHLF:(k + 1) * NHLF])
                        return go
                    for k in range(KT):
                        q.append(mk_t(k))
                st = {}

                def sums():
                    st["s"] = ps_ln.tile([1, NHLF], F32, tag="lns",
                                         name="ln_s")[:]
                    st["q"] = ps_ln.tile([1, NHLF], F32, tag="lns",
                                         name="ln_q")[:]
                    for k in range(KT):
                        tk = tmp[:, k * NHLF:(k + 1) * NHLF]
                        sq = sm.tile([128, NHLF], F32, tag="lnsq", bufs=3)
                        nc.scalar.activation(sq[:], tk, AF.Square)
                        nc.tensor.matmul(st["s"], ones_col[:], tk,
                                         start=(k == 0), stop=(k == KT - 1))
                        nc.tensor.matmul(st["q"], ones_col[:], sq[:],
                                         start=(k == 0), stop=(k == KT - 1))

                q.append(sums)

                def finishA():
                    murd = sm.tile([1, 2 * NHLF], F32, tag="ln1", bufs=2)
                    mu = murd[:, :NHLF]
                    rstd = murd[:, NHLF:]
                    nc.vector.tensor_scalar_mul(mu, st["s"], 1.0 / D)
                    mu2 = sm.tile([1, NHLF], F32, tag="ln2", bufs=2)
                    nc.vector.tensor_mul(mu2[:], mu, mu)
                    var = sm.tile([1, NHLF], F32, tag="ln3", bufs=2)
                    nc.vector.scalar_tensor_tensor(var[:], st["q"], 1.0 / D,
                                                   mu2[:], op0=ALU.mult,
                                                   op1=ALU.subtract)
                    sd = sm.tile([1, NHLF], F32, tag="ln4", bufs=2)
                    nc.scalar.activation(sd[:], var[:], AF.Sqrt, bias=epsb[:])
                    nc.vector.reciprocal(rstd, sd[:])
                    st["murd"] = murd

                def finishB():
                    bc = ps_bc.tile([128, 2 * NHLF], F32, tag="bc")
                    nc.tensor.matmul(bc[:], ones_row[:], st["murd"][:],
                                     start=True, stop=True)
                    st["bc"] = bc

                def mk_norm(k):
                    def go():
                        mu_bc = st["bc"][:, :NHLF]
                        rstd_bc = st["bc"][:, NHLF:]
                        tk = tmp[:, k * NHLF:(k + 1) * NHLF]
                        u = sm.tile([128, NHLF], F32, tag="lnu", bufs=2)
                        nc.vector.scalar_tensor_tensor(u[:], tk, 1.0, mu_bc,
                                                       op0=ALU.mult,
                                                       op1=ALU.subtract)
                        nc.vector.tensor_mul(u[:], u[:], rstd_bc)
                        nc.vector.tensor_scalar(xs(k, hs), u[:],
                                                w_ap[:, k:k + 1],
                                                b_ap[:, k:k + 1],
                                                op0=ALU.mult, op1=ALU.add)
                        if make16:
                            nc.scalar.activation(xs16(k, hs), xs(k, hs),
                                                 AF.Identity)
                    return go

                q.append(finishA)
                q.append(finishB)
                for k in range(KT):
                    q.append(mk_norm(k))

            def load_attn_weights(l):
                aw = {}
                aw["wq"] = awpool.tile([128, KT * H], F16, name="wq", tag="wq")
                aw["wk"] = awpool.tile([128, KT * H], F16, name="wk", tag="wk")
                aw["wv"] = awpool.tile([128, KT * H], F16, name="wv", tag="wv")
                nc.sync.dma_start(aw["wq"][:].rearrange("p (k h) -> p k h",
                                                        k=KT), d_wqT[l])
                nc.sync.dma_start(aw["wk"][:].rearrange("p (k h) -> p k h",
                                                        k=KT), d_wkT[l])
                nc.sync.dma_start(aw["wv"][:].rearrange("p (k h) -> p k h",
                                                        k=KT), d_wvT[l])
                aw["wp"] = awpool.tile([128, KT * D], F16, name="wp", tag="wp")
                nc.sync.dma_start(aw["wp"][:].rearrange("p (k d) -> p k d",
                                                        k=KT), d_wpT[l])
                aw["bpj"] = awpool.tile([128, KT], F32, name="bpj", tag="bpj")
                nc.sync.dma_start(aw["bpj"][:], d_bproj[l])
                aw["l1w"] = awpool.tile([128, KT], F32, name="l1w", tag="l1w")
                nc.sync.dma_start(aw["l1w"][:], d_ln1w[l])
                aw["l1b"] = awpool.tile([128, KT], F32, name="l1b", tag="l1b")
                nc.sync.dma_start(aw["l1b"][:], d_ln1b[l])
                return aw

            def load_moe_weights(l):
                mw = {}
                mw["gw"] = awpool.tile([128, KT * E], F32, name="gw", tag="gw")
                nc.sync.dma_start(mw["gw"][:].rearrange("p (k e) -> p k e",
                                                        k=KT), d_gwT[l])
                mw["gb"] = awpool.tile([1, E], F32, name="gb", tag="gb")
                nc.sync.dma_start(mw["gb"][:], d_gb[l])
                mw["l2w"] = awpool.tile([128, KT], F32, name="l2w", tag="l2w")
                nc.sync.dma_start(mw["l2w"][:], d_ln2w[l])
                mw["l2b"] = awpool.tile([128, KT], F32, name="l2b", tag="l2b")
                nc.sync.dma_start(mw["l2b"][:], d_ln2b[l])
                mw["b1t"] = awpool.tile([128, MT], F32, name="b1t", tag="b1t")
                nc.sync.dma_start(mw["b1t"][:], d_b1[l])
                mw["b2own"] = awpool.tile([128, KT], F32, name="b2own",
                                          tag="b2t")
                nc.sync.dma_start(mw["b2own"][:], d_b2c[l])
                return mw

            def attn_phase(l, h, aw):
                """This core's head for half h -> o fp16 -> AllGather."""
                hs = hslice(h)
                q_ps = ps_acc.tile([H, NHLF], F32, tag="acc")
                k_ps = ps_acc.tile([H, NHLF], F32, tag="acc")
                v_ps = ps_acc.tile([H, NHLF], F32, tag="acc")
                for k in range(KT):
                    nc.tensor.matmul(q_ps[:], aw["wq"][:, k * H:(k + 1) * H],
                                     xs16(k, hs), start=(k == 0),
                                     stop=(k == KT - 1))
                for k in range(KT):
                    nc.tensor.matmul(k_ps[:], aw["wk"][:, k * H:(k + 1) * H],
                                     xs16(k, hs), start=(k == 0),
                                     stop=(k == KT - 1))
                for k in range(KT):
                    nc.tensor.matmul(v_ps[:], aw["wv"][:, k * H:(k + 1) * H],
                                     xs16(k, hs), start=(k == 0),
                                     stop=(k == KT - 1))
                qT = wk.tile([H, NHLF], F32, name=f"qT{h}", bufs=1)
                kT_ = wk.tile([H, NHLF], F32, name=f"kT{h}", bufs=1)
                vT = wk.tile([H, NHLF], F32, name=f"vT{h}", bufs=1)
                nc.scalar.activation(qT[:], q_ps[:], AF.Identity)
                nc.scalar.activation(kT_[:], k_ps[:], AF.Identity)
                nc.scalar.activation(vT[:], v_ps[:], AF.Identity)

                oT = wk.tile([H, NHLF], F16, name=f"oT{h}", bufs=1)
                w4 = ps_att.tile([64, NHLF], F32, tag="a", name="w4")
                for b in range(4):
                    ts_ = slice(b * 64, (b + 1) * 64)
                    nc.tensor.matmul(w4[:, ts_], qT[:, ts_], kT_[:, ts_],
                                     start=True, stop=True)
                s4 = sm.tile([64, NHLF], F32, tag="att_s", bufs=1)
                nc.vector.scalar_tensor_tensor(s4[:], w4[:], SCALE,
                                               mask4[:], op0=ALU.mult,
                                               op1=ALU.add)
                ssum4 = sm.tile([64, 4], F32, tag="att_su", bufs=1)
                rs4 = sm.tile([64, 4], F32, tag="att_r", bufs=1)
                att4 = sm.tile([64, NHLF], F32, tag="att_a", bufs=1)
                for b in range(4):
                    ts_ = slice(b * 64, (b + 1) * 64)
                    mx = sm.tile([64, 1], F32, tag="att_m", bufs=2)
                    nc.vector.reduce_max(mx[:], s4[:, ts_], axis=AX.X,
                                         negate=True)
                    nc.scalar.activation(att4[:, ts_], s4[:, ts_], AF.Exp,
                                         bias=mx[:],
                                         accum_out=ssum4[:, b:b + 1])
                nc.vector.reciprocal(rs4[:], ssum4[:])
                for b in range(4):
                    ts_ = slice(b * 64, (b + 1) * 64)
                    nc.vector.tensor_scalar_mul(att4[:, ts_], att4[:, ts_],
                                                rs4[:, b:b + 1])
                at4 = ps_att.tile([64, NHLF], F32, tag="a", name="at4")
                for b in range(4):
                    ts_ = slice(b * 64, (b + 1) * 64)
                    nc.tensor.transpose(at4[:, ts_], att4[:, ts_],
                                        ident[:64, :64])
                attT4 = sm.tile([64, NHLF], F32, tag="att_t", bufs=1)
                nc.scalar.activation(attT4[:], at4[:], AF.Identity)
                vt4 = ps_att.tile([64, 4 * H], F32, tag="a", name="vt4")
                for b in range(4):
                    nc.tensor.transpose(vt4[:, b * H:(b + 1) * H],
                                        vT[:, b * 64:(b + 1) * 64],
                                        ident[:H, :H])
                vtb4 = sm.tile([64, 4 * H], F32, tag="att_v", bufs=1)
                nc.vector.tensor_copy(vtb4[:], vt4[:])
                o4 = ps_att.tile([H, NHLF], F32, tag="a", name="o4")
                for b in range(4):
                    ts_ = slice(b * 64, (b + 1) * 64)
                    nc.tensor.matmul(o4[:, ts_],
                                     vtb4[:, b * H:(b + 1) * H],
                                     attT4[:, ts_], start=True, stop=True)
                nc.scalar.activation(oT[:], o4[:], AF.Identity)

                nc.sync.dma_start(ag_o_i[l][h][:], oT[:])
                nc.gpsimd.collective_compute(
                    "AllGather", ALU.bypass,
                    replica_groups=[list(range(N_CORES))],
                    ins=[ag_o_i[l][h][:]], outs=[ag_o_o[l][h][:]])

            def proj_ln1_gate_emit(q, l, h, aw, mw):
                """Append chunks: o16 load, 6 proj, 13 LN, 5 gate. Returns
                (combT, cbc, comb16) tiles filled when chunks run."""
                hs = hslice(h)
                o16 = wk.tile([128, KT * NHLF], F16, name=f"o16{h}", bufs=1)
                t1 = wk.tile([128, KT * NHLF], F32, name=f"t1_{h}", bufs=1)
                combT = sm.tile([E, NHLF], F32, name=f"combT{h}", bufs=1)
                cbc = wk.tile([128, NHLF], F32, name=f"cbc{h}", bufs=1)

                def load_o():
                    for k in range(KT):
                        nc.sync.dma_start(o16[:, k * NHLF:(k + 1) * NHLF],
                                          ag_o_o[l][h][k * 128:(k + 1) * 128, :])
                q.append(load_o)

                def mk_proj(m):
                    def go():
                        y_ps = ps_acc.tile([128, NHLF], F32, tag="acc")
                        for k in range(KT):
                            nc.tensor.matmul(
                                y_ps[:],
                                aw["wp"][:, k * D + m * 128:
                                         k * D + (m + 1) * 128],
                                o16[:, k * NHLF:(k + 1) * NHLF],
                                start=(k == 0), stop=(k == KT - 1))
                        # t = (y + bproj) + x, read straight from PSUM
                        nc.vector.scalar_tensor_tensor(
                            t1[:, m * NHLF:(m + 1) * NHLF], y_ps[:],
                            aw["bpj"][:, m:m + 1], xs(m, hs),
                            op0=ALU.add, op1=ALU.add)
                    return go
                for m in range(KT):
                    q.append(mk_proj(m))

                layernorm_emit(q, h, None, aw["l1w"], aw["l1b"], True,
                               f"1_{h}", tmp=t1)

                g_pss = []

                def mk_glog(tt):
                    def go():
                        g_ps = ps_att.tile([128, E], F32, tag="a", name="g_ps")
                        for k in range(KT):
                            nc.tensor.matmul(
                                g_ps[:],
                                x_sb[:, k * N + hs.start + tt * 128:
                                     k * N + hs.start + (tt + 1) * 128],
                                mw["gw"][:, k * E:(k + 1) * E],
                                start=(k == 0), stop=False)
                        nc.tensor.matmul(g_ps[:], ones_row[:], mw["gb"][:],
                                         start=False, stop=True)
                        g_pss.append(g_ps)
                    return go

                cws = []

                def mk_smax(tt):
                    def go():
                        g_ps = g_pss[tt]
                        mx = sm.tile([128, 1], F32, tag="g_m")
                        nc.vector.reduce_max(mx[:], g_ps[:], axis=AX.X,
                                             negate=True)
                        pr = sm.tile([128, E], F32, tag="g_p")
                        ssum = sm.tile([128, 1], F32, tag="g_s")
                        nc.scalar.activation(pr[:], g_ps[:], AF.Exp, bias=mx[:],
                                             accum_out=ssum[:])
                        rs = sm.tile([128, 1], F32, tag="g_r")
                        nc.vector.reciprocal(rs[:], ssum[:])
                        nc.vector.tensor_scalar_mul(pr[:], pr[:], rs[:])
                        top8 = sm.tile([128, 8], F32, tag="g_t8")
                        nc.vector.max(out=top8[:], in_=pr[:])
                        msk = sm.tile([128, E], F32, tag="g_msk")
                        nc.vector.tensor_scalar(msk[:], pr[:], top8[:, 1:2],
                                                None, op0=ALU.is_ge)
                        cw = sm.tile([128, E], F32, tag=f"g_cw{tt}", bufs=1)
                        nc.vector.tensor_mul(cw[:], pr[:], msk[:])
                        den = sm.tile([128, 1], F32, tag="g_den")
                        nc.vector.tensor_add(den[:], top8[:, 0:1], top8[:, 1:2])
                        dr = sm.tile([128, 1], F32, tag="g_dr")
                        nc.vector.reciprocal(dr[:], den[:])
                        nc.vector.tensor_scalar_mul(cw[:], cw[:], dr[:])
                        cws.append(cw)
                    return go

                def crow_cbc():
                    for tt in range(2):
                        ct_ps = ps_att.tile([E, 128], F32, tag="a", name="ct_ps")
                        nc.tensor.transpose(ct_ps[:], cws[tt][:], ident[:])
                        nc.vector.tensor_copy(
                            combT[:, tt * 128:(tt + 1) * 128], ct_ps[:])
                    crow_ps = ps_att.tile([1, NHLF], F32, tag="a", name="crow_ps")
                    nc.tensor.matmul(crow_ps[:], combsel[:], combT[:],
                                     start=True, stop=True)
                    crow = sm.tile([1, NHLF], F32, tag=f"crow{h}", bufs=1)
                    nc.vector.tensor_copy(crow[:], crow_ps[:])
                    cbc_ps = ps_bc.tile([128, NHLF], F32, tag="bc")
                    nc.tensor.matmul(cbc_ps[:], ones_row[:], crow[:],
                                     start=True, stop=True)
                    nc.vector.tensor_copy(cbc[:], cbc_ps[:])

                q.append(mk_glog(0))
                q.append(mk_glog(1))
                q.append(mk_smax(0))
                q.append(mk_smax(1))
                q.append(crow_cbc)
                return combT, cbc

            def ffn_phase(l, h, mw, cbc, q, w1_slots=True):
                """Dense expert FFN for half h (fp16), -> AllReduce.

                Pops one pending chunk from q after each w1 m-tile (if
                w1_slots) and two after each w2 m-tile."""
                hs = hslice(h)
                hT = hpool.tile([128, MT * NHLF], F16, tag="hT")

                def fill(n):
                    for _ in range(n):
                        if q:
                            f = q.pop(0)
                            if f is not None:
                                f()

                for m in range(MT):
                    w1m = w1pool.tile([128, KT * 128], F16, tag="w1")
                    nc.sync.dma_start(
                        w1m[:].rearrange("p (k f) -> p k f", k=KT),
                        d_w1h[l, m])
                    h_ps = ps_acc.tile([128, NHLF], F32, tag="acc")
                    for k in range(KT):
                        nc.tensor.matmul(
                            h_ps[:], w1m[:, k * 128:(k + 1) * 128],
                            xs16(k, hs),
                            start=(k == 0), stop=(k == KT - 1))
                    nc.scalar.activation(
                        hT[:, m * NHLF:(m + 1) * NHLF],
                        h_ps[:], AF.Gelu, bias=mw["b1t"][:, m:m + 1])
                    if w1_slots:
                        fill(1)
                for m in range(KT):
                    w2m = w2pool.tile([128, MT * 128], F16, tag="w2")
                    nc.sync.dma_start(
                        w2m[:].rearrange("p (j f) -> p j f", j=MT),
                        d_w2h[l, m])
                    ye_ps = ps_acc.tile([128, NHLF], F32, tag="acc")
                    for j in range(MT):
                        nc.tensor.matmul(
                            ye_ps[:], w2m[:, j * 128:(j + 1) * 128],
                            hT[:, j * NHLF:(j + 1) * NHLF],
                            start=(j == 0), stop=(j == MT - 1))
                    ysc = sm.tile([128, NHLF], F16, tag="ycp", bufs=2)
                    nc.vector.scalar_tensor_tensor(
                        ysc[:], ye_ps[:], mw["b2own"][:, m:m + 1], cbc[:],
                        op0=ALU.add, op1=ALU.mult)
                    nc.sync.dma_start(
                        ar_ffn_i[l][h][m * 128:(m + 1) * 128, :], ysc[:])
                    fill(3)
                nc.gpsimd.collective_compute(
                    "AllReduce", ALU.add,
                    replica_groups=[list(range(N_CORES))],
                    ins=[ar_ffn_i[l][h][:]], outs=[ar_ffn_o[l][h][:]])

            def ln2_emit(q, l, h, mw, combT, make16):
                """Append chunks: ym16 load + LN2 (residual + b2@comb)."""
                ym16 = wk.tile([128, KT * NHLF], F16, name=f"ym16{h}", bufs=1)
                hs = hslice(h)

                def load_ym():
                    for k in range(KT):
                        nc.sync.dma_start(ym16[:, k * NHLF:(k + 1) * NHLF],
                                          ar_ffn_o[l][h][k * 128:(k + 1) * 128,
                                                         :])
                q.append(load_ym)

                def get_t2(k, dst):
                    nc.vector.tensor_add(dst, ym16[:, k * NHLF:(k + 1) * NHLF],
                                         xs(k, hs))

                layernorm_emit(q, h, get_t2, mw["l2w"], mw["l2b"], make16,
                               f"2_{h}")

            def final_phase(h):
                """Final layernorm + lm head for half h."""
                hs = hslice(h)

                def get_tf(k, dst):
                    nc.vector.tensor_copy(dst, xs(k, hs))

                qf = []
                layernorm_emit(qf, h, get_tf, lnfw, lnfb, True, f"f_{h}")
                for f in qf:
                    f()
                lg_ps = ps_acc.tile([V, NHLF], F32, tag="acc")
                for k in range(KT):
                    nc.tensor.matmul(
                        lg_ps[:], lmT[:, k * V:(k + 1) * V],
                        xs16(k, hs),
                        start=(k == 0), stop=(k == KT - 1))
                lg = sm.tile([V, NHLF], F32, tag=f"lgout{h}", bufs=1)
                nc.scalar.activation(lg[:], lg_ps[:], AF.Identity, bias=lmb[:])
                nc.sync.dma_start(d_out[:, hs], lg[:])

            # ---- software-pipelined layer loop ----
            aw = load_attn_weights(0)
            attn_phase(0, 0, aw)
            attn_phase(0, 1, aw)
            for l in range(L):
                mw = load_moe_weights(l)
                q0 = []
                combT0, cbc0 = proj_ln1_gate_emit(q0, l, 0, aw, mw)
                for f in q0:
                    f()
                if l < L - 1:
                    aw2 = load_attn_weights(l + 1)
                q1 = []
                combT1, cbc1 = proj_ln1_gate_emit(q1, l, 1, aw, mw)
                ffn_phase(l, 0, mw, cbc0, q1)
                for f in q1:
                    f()
                # pad so ln2 chunks pop only once AR(l,0) has landed
                # (~20us after its trigger at ffn(l,0) end); the ym16 DMA
                # then never stalls the sync queue ahead of w1 prefetch.
                q2 = [None] * 20
                ln2_emit(q2, l, 0, mw, combT0,
                         make16=(l < L - 1))
                ffn_phase(l, 1, mw, cbc1, q2, w1_slots=True)
                for f in q2:
                    if f is not None:
                        f()
                if l < L - 1:
                    attn_phase(l + 1, 0, aw2)
                    q3 = []
                    ln2_emit(q3, l, 1, mw, combT1, make16=True)
                    for f in q3:
                        f()
                    attn_phase(l + 1, 1, aw2)
                    aw = aw2
                else:
                    final_phase(0)
                    q3 = []
                    ln2_emit(q3, l, 1, mw, combT1, make16=False)
                    for f in q3:
                        f()
                    final_phase(1)

    return nc


def _prep(inputs):
    """Build per-core input maps from the full input dict."""
    f = lambda a: np.ascontiguousarray(np.asarray(a), dtype=np.float32)
    h = lambda a: np.ascontiguousarray(np.asarray(a), dtype=np.float16)
    idx = np.asarray(inputs["idx"]).reshape(1, N)
    wq, wkk, wv = f(inputs["wq"]), f(inputs["wk"]), f(inputs["wv"])
    wproj, bproj = f(inputs["wproj"]), f(inputs["bproj"])
    gate_w, gate_b = f(inputs["gate_w"]), f(inputs["gate_b"])
    w1, b1 = f(inputs["w1"]), f(inputs["b1"])
    w2, b2 = f(inputs["w2"]), f(inputs["b2"])

    base = {
        "idx": np.ascontiguousarray(idx.astype(np.int32)),
        "iota99": np.arange(V, dtype=np.float32).reshape(V, 1),
        "ident128": np.eye(128, dtype=np.float32),
        "maskb": np.where(np.tril(np.ones((64, 64), bool)), 0.0,
                          -1e30).astype(np.float32),
        "ones_col": np.ones((128, 1), np.float32),
        "ones_row": np.ones((1, 128), np.float32),
        "tok_emb": f(inputs["tok_emb"]),
        "posT": np.ascontiguousarray(
            np.tile(f(inputs["pos_emb"]).T, (1, B))),
        # full wproj^T for every core: [in 768 -> KT,128][out 768]
        "wpT": np.ascontiguousarray(
            wproj.transpose(0, 2, 1).reshape(L, KT, 128, D)
            .transpose(0, 2, 1, 3)).astype(np.float16),
        "gwT": np.ascontiguousarray(
            gate_w.transpose(0, 2, 1).reshape(L, KT, 128, E)
            .transpose(0, 2, 1, 3)),
        "gb": gate_b.reshape(L, 1, E),
        
        "ln1w": np.ascontiguousarray(f(inputs["ln1_w"]).reshape(L, KT, 128).transpose(0, 2, 1)),
        "ln1b": np.ascontiguousarray(f(inputs["ln1_b"]).reshape(L, KT, 128).transpose(0, 2, 1)),
        "ln2w": np.ascontiguousarray(f(inputs["ln2_w"]).reshape(L, KT, 128).transpose(0, 2, 1)),
        "ln2b": np.ascontiguousarray(f(inputs["ln2_b"]).reshape(L, KT, 128).transpose(0, 2, 1)),
        "lnfw": np.ascontiguousarray(f(inputs["lnf_w"]).reshape(KT, 128).T),
        "lnfb": np.ascontiguousarray(f(inputs["lnf_b"]).reshape(KT, 128).T),
        "lmT": np.ascontiguousarray(
            f(inputs["lm_w"]).T.reshape(KT, 128, V)).astype(np.float16),
        "dar": np.zeros((1, 8), np.float32),
        "epsb": np.full((1, 1), EPS, np.float32),
        "lmb": f(inputs["lm_b"]).reshape(V, 1),
        "bproj": np.ascontiguousarray(bproj.reshape(L, KT, 128).transpose(0, 2, 1)),
    }
    in_maps = []
    for c in range(N_CORES):
        m = dict(base)
        m["wqT"] = np.ascontiguousarray(
            wq[:, c].transpose(0, 2, 1).reshape(L, KT, 128, H)
            .transpose(0, 2, 1, 3)).astype(np.float16)
        m["wkT"] = np.ascontiguousarray(
            wkk[:, c].transpose(0, 2, 1).reshape(L, KT, 128, H)
            .transpose(0, 2, 1, 3)).astype(np.float16)
        m["wvT"] = np.ascontiguousarray(
            wv[:, c].transpose(0, 2, 1).reshape(L, KT, 128, H)
            .transpose(0, 2, 1, 3)).astype(np.float16)
        w1tc = w1[:, c].transpose(0, 2, 1)  # [L, 768, 3072]
        m["w1h"] = np.ascontiguousarray(
            w1tc.reshape(L, KT, 128, MT, 128).transpose(0, 3, 2, 1, 4)).astype(
                np.float16)
        m["b1"] = np.ascontiguousarray(b1[:, c].reshape(L, MT, 128).transpose(0, 2, 1))
        w2tc = w2[:, c].transpose(0, 2, 1)  # [L, 3072, 768]
        m["w2h"] = np.ascontiguousarray(
            w2tc.reshape(L, MT, 128, KT, 128).transpose(0, 3, 2, 1, 4)).astype(
                np.float16)
        m["b2own"] = np.ascontiguousarray(
            b2[:, c].reshape(L, KT, 128).transpose(0, 2, 1))
        sel = np.zeros((E, 1), np.float32)
        sel[c, 0] = 1.0
        m["combsel"] = sel
        in_maps.append(m)
    return in_maps


def kernel(**inputs) -> np.ndarray:
    if "nc" not in _CACHED:
        _CACHED["nc"] = build()
    nc = _CACHED["nc"]
    in_maps = _prep(inputs)
    res = run_bass_kernel_spmd(nc, in_maps, list(range(N_CORES)))
    lt = res.results[0]["logitsT"]  # [V, N]
    return np.ascontiguousarray(lt.T.reshape(B, T, V).astype(np.float32))


if __name__ == "__main__":
    import jax

    jax.config.update("jax_platforms", "cpu")
    import reference as ref

    inp = ref.setup_inputs()
    want = np.asarray(ref.reference(**inp))
    import jax as _j
    _j.config.update("jax_platforms", "axon")
    got = kernel(**{k: np.asarray(v) for k, v in inp.items()})
    err = np.abs(got - want).max()
    rel = err / np.abs(want).max()
    l2 = np.linalg.norm(got - want) / np.linalg.norm(want)
    print(f"absmax {err:.3e}  absmax-rel {rel:.3e}  l2-rel {l2:.3e}")


# revision 9
# speedup vs baseline: 1.0126x; 1.0126x over previous
"""Trainium2 Bass kernel for nn_CustomTransformer_58445914964311 (v3).

12-layer MoE transformer (768 embd, 8 heads, 8 experts top-2, B=8 x T=64
tokens), distributed over 8 NeuronCores:
  - attention sharded by head (core c computes head c for all tokens);
    per-head outputs o are AllGathered in fp16 (~7us) and the full output
    projection is computed redundantly on every core, replacing the
    baseline's AllReduce of projection partials.
  - MoE sharded by expert (core c computes expert c densely over all 512
    tokens -- the top-2 routing is too imbalanced to exploit, counts per
    (layer, expert) range 3..426 of 512). Partials (including the b2
    expert bias, folded in pre-AllReduce) are AllReduced in fp16 (~20us
    per 256-token half).
  - the residual stream, layernorms, softmaxes and gate logits stay
    fp32: gate top-2 margins go down to 2.6e-6 and even 1e-5-level
    stream noise flips expert choices (measured: fp16 LN sum-of-squares
    or fp16 b2 bias each cost ~0.25 abs rel-err). All heavy matmuls
    (qkv, proj, w1, w2, lm head) run in fp16 at 1 cyc/row.

Schedule: two 256-token halves (batch rows 0-3 / 4-7; causal attention
never crosses rows). Each half's FFN weight-stream is interleaved at
m-tile granularity with the other half's projection/LN/gate (or ln2)
chunks so the PE queue stays backlogged while vector/scalar chains
drain; AllReduces ride under the other half's FFN. Weight tensors are
laid out host-side so every DMA is contiguous per partition line
(256-byte strided tile loads starved the PE at m-tile boundaries).

Perf history on this fabric: fp32 baseline 5.47ms -> fp16 + o-AllGather
3.14ms -> contiguous weight DMA 2.82ms -> b2-fold + batched LN sums
2.74ms -> batched attention/gate + pool rebalance 2.60ms -> deeper
w1/w2 prefetch + earlier next-layer weight loads + denser w2-phase
filler slots 2.54ms. Run-to-run variance on this fabric is ~5-10%.

Self-contained: hardcodes all shapes; host side only reshapes/casts.
"""

import numpy as np

import concourse.bass as bass
import concourse.mybir as mybir
import concourse.tile as tile
from concourse.bass_utils import run_bass_kernel_spmd

import os
import sys

# ---------------------------------------------------------------------------
# Compatibility patches (inlined): the walrus build here rejects instructions
# carrying more than one semaphore wait ("Too many sync wait commands").
# ---------------------------------------------------------------------------
import orjson as _orjson
from concourse.vector_clock import ScopedClock as _ScopedClock

_COMPAT_DONE = False


def _patched_drain_and_barrier(self, tick_clock, wait_clock):
    nc = self.nc
    collector = nc.sync.nop()
    wait_clock.add_sem_waits(
        collector.ins, _ScopedClock({None: tick_clock.global_clock})
    )
    si = collector.ins.sync_info
    waits = list(si.on_wait or []) if si is not None else []
    if len(waits) > 1:
        si.on_wait = waits[:1]
        for w in waits[1:]:
            extra = nc.sync.nop()
            esi = extra.ins.sync_info
            if esi is None:
                extra.ins.sync_info = mybir.SyncInfo(on_wait=[w], on_update=[])
            else:
                esi.on_wait = [w]
    nc.sync.drain()
    nc.all_engine_barrier()
    popped = nc._tile_sem_poison_stack.pop()
    assert popped is self._sem_poison
    nc.clear_and_free_semaphores(list(self.sems.allocated().values()))
    nc.all_engine_barrier()


def _split_multi_waits(mod, max_waits=1):
    ctr = 0
    for fn in mod.get("functions", []):
        for blk in fn.get("blocks", []):
            insts = blk.get("instructions", [])
            if not any(
                len((i.get("sync_info") or {}).get("on_wait") or []) > max_waits
                for i in insts
            ):
                continue
            new_insts = []
            for inst in insts:
                si = inst.get("sync_info")
                waits = (si.get("on_wait") or []) if si else []
                if len(waits) > max_waits:
                    for w in waits[max_waits:]:
                        ctr += 1
                        new_insts.append({
                            "debug": inst.get("debug", 0),
                            "engine": inst["engine"],
                            "ins": [], "outs": [],
                            "name": f"{inst['name']}-wsp{ctr}",
                            "opcode": "EventSemaphore",
                            "sync_info": {"on_update": [], "on_wait": [w]},
                        })
                    si["on_wait"] = waits[:max_waits]
                new_insts.append(inst)
            blk["instructions"] = new_insts
    return mod


_orig_to_json_bytes = bass.Bass.to_json_bytes


def _patched_to_json_bytes(self):
    return _orjson.dumps(_split_multi_waits(_orjson.loads(_orig_to_json_bytes(self))))


def _install_ntff_hook_shim():
    import types
    if "antenv.axon_hooks" in sys.modules:
        return
    try:
        import antenv  # noqa: F401
    except ImportError:
        return
    mod = types.ModuleType("antenv.axon_hooks")
    _state = {"hook": None}
    mod.set_axon_ntff_profile_hook = lambda hook: _state.__setitem__("hook", hook)
    mod.get_axon_ntff_profile_hook = lambda: _state["hook"]
    sys.modules["antenv.axon_hooks"] = mod
    sys.modules["antenv"].axon_hooks = mod
    try:
        from trn_agent_boot.trn_boot import _ntff_profile_via_ctypes
        hook = _ntff_profile_via_ctypes("/opt/axon/libaxon_pjrt.so")
        if hook is not None:
            mod.set_axon_ntff_profile_hook(hook)
    except Exception:
        pass


def _install_compat():
    global _COMPAT_DONE
    if _COMPAT_DONE:
        return
    tile.TileContext._drain_and_barrier = _patched_drain_and_barrier
    bass.Bass.to_json_bytes = _patched_to_json_bytes
    _install_ntff_hook_shim()
    _COMPAT_DONE = True


_install_compat()

F32 = mybir.dt.float32
F16 = mybir.dt.float16
I32 = mybir.dt.int32
AF = mybir.ActivationFunctionType
ALU = mybir.AluOpType
AX = mybir.AxisListType

N_CORES = 8
L = 12
D = 768
H = 96          # head dim
NH = 8
E = 8           # experts
DFF = 3072
B, T = 8, 64
N = B * T       # 512 tokens
NHLF = N // 2   # 256 tokens per pipeline half
V = 99
KT = D // 128   # 6 feature tiles
MT = DFF // 128  # 24 dff tiles
EPS = 1e-5
SCALE = H ** -0.5

_CACHED = {}


def build():
    nc = bass.Bass(num_devices=N_CORES)

    # ---- inputs (per-core data, same names) ----
    d_idx = nc.dram_tensor("idx", [1, N], I32, kind="ExternalInput")
    d_iota = nc.dram_tensor("iota99", [V, 1], F32, kind="ExternalInput")
    d_ident = nc.dram_tensor("ident128", [128, 128], F32, kind="ExternalInput")
    d_mask = nc.dram_tensor("maskb", [64, 64], F32, kind="ExternalInput")
    d_ones_col = nc.dram_tensor("ones_col", [128, 1], F32, kind="ExternalInput")
    d_ones_row = nc.dram_tensor("ones_row", [1, 128], F32, kind="ExternalInput")
    d_tok = nc.dram_tensor("tok_emb", [V, D], F32, kind="ExternalInput")
    d_posT = nc.dram_tensor("posT", [D, N], F32, kind="ExternalInput")
    d_wqT = nc.dram_tensor("wqT", [L, 128, KT, H], F16, kind="ExternalInput")
    d_wkT = nc.dram_tensor("wkT", [L, 128, KT, H], F16, kind="ExternalInput")
    d_wvT = nc.dram_tensor("wvT", [L, 128, KT, H], F16, kind="ExternalInput")
    d_wpT = nc.dram_tensor("wpT", [L, 128, KT, D], F16, kind="ExternalInput")
    d_bproj = nc.dram_tensor("bproj", [L, 128, KT], F32, kind="ExternalInput")
    d_gwT = nc.dram_tensor("gwT", [L, 128, KT, E], F32, kind="ExternalInput")
    d_gb = nc.dram_tensor("gb", [L, 1, E], F32, kind="ExternalInput")
    d_b1 = nc.dram_tensor("b1", [L, 128, MT], F32, kind="ExternalInput")
    d_b2c = nc.dram_tensor("b2own", [L, 128, KT], F32, kind="ExternalInput")
    d_combsel = nc.dram_tensor("combsel", [E, 1], F32, kind="ExternalInput")
    d_ln1w = nc.dram_tensor("ln1w", [L, 128, KT], F32, kind="ExternalInput")
    d_ln1b = nc.dram_tensor("ln1b", [L, 128, KT], F32, kind="ExternalInput")
    d_ln2w = nc.dram_tensor("ln2w", [L, 128, KT], F32, kind="ExternalInput")
    d_ln2b = nc.dram_tensor("ln2b", [L, 128, KT], F32, kind="ExternalInput")
    d_lnfw = nc.dram_tensor("lnfw", [128, KT], F32, kind="ExternalInput")
    d_lnfb = nc.dram_tensor("lnfb", [128, KT], F32, kind="ExternalInput")
    d_lmT = nc.dram_tensor("lmT", [KT, 128, V], F16, kind="ExternalInput")
    d_w1h = nc.dram_tensor("w1h", [L, MT, 128, KT, 128], F16, kind="ExternalInput")
    d_w2h = nc.dram_tensor("w2h", [L, KT, 128, MT, 128], F16, kind="ExternalInput")
    d_dar = nc.dram_tensor("dar", [1, 8], F32, kind="ExternalInput")
    d_epsb = nc.dram_tensor("epsb", [1, 1], F32, kind="ExternalInput")
    d_lmb = nc.dram_tensor("lmb", [V, 1], F32, kind="ExternalInput")
    d_out = nc.dram_tensor("logitsT", [V, N], F32, kind="ExternalOutput")

    with tile.TileContext(nc) as tc:
        with (
            tc.tile_pool(name="const", bufs=1) as cpool,
            tc.tile_pool(name="x", bufs=1) as xpool,
            tc.tile_pool(name="attw", bufs=2) as awpool,
            tc.tile_pool(name="w1", bufs=10) as w1pool,
            tc.tile_pool(name="w2", bufs=5) as w2pool,
            tc.tile_pool(name="h", bufs=1) as hpool,
            tc.tile_pool(name="work", bufs=2) as wk,
            tc.tile_pool(name="small", bufs=3) as sm,
            tc.tile_pool(name="ps_acc", bufs=3, space="PSUM") as ps_acc,
            tc.tile_pool(name="ps_bc", bufs=1, space="PSUM") as ps_bc,
            tc.tile_pool(name="ps_ln", bufs=2, space="PSUM") as ps_ln,
            tc.tile_pool(name="ps_att", bufs=2, space="PSUM") as ps_att,
            tc.tile_pool(name="dram", bufs=1, space="DRAM") as dpool,
        ):
            # ---- constants resident ----
            ident = cpool.tile([128, 128], F32, name="ident")
            nc.sync.dma_start(ident[:], d_ident[:])
            maskb = cpool.tile([64, 64], F32, name="maskb")
            nc.sync.dma_start(maskb[:], d_mask[:])
            mask4 = cpool.tile([64, NHLF], F32, name="mask4")
            for _b in range(4):
                nc.vector.tensor_copy(mask4[:, _b * 64:(_b + 1) * 64],
                                      maskb[:])
            iota99 = cpool.tile([V, 1], F32, name="iota99")
            nc.sync.dma_start(iota99[:], d_iota[:])
            ones_col = cpool.tile([128, 1], F32, name="ones_col")
            nc.sync.dma_start(ones_col[:], d_ones_col[:])
            ones_row = cpool.tile([1, 128], F32, name="ones_row")
            nc.sync.dma_start(ones_row[:], d_ones_row[:])
            combsel = cpool.tile([E, 1], F32, name="combsel")
            nc.sync.dma_start(combsel[:], d_combsel[:])
            epsb = cpool.tile([1, 1], F32, name="epsb")
            nc.sync.dma_start(epsb[:], d_epsb[:])
            tok = cpool.tile([V, D], F32, name="tok")
            nc.sync.dma_start(tok[:], d_tok[:])
            lmT = cpool.tile([128, KT * V], F16, name="lmT")
            for k in range(KT):
                nc.sync.dma_start(lmT[:, k * V:(k + 1) * V], d_lmT[k])
            lmb = cpool.tile([V, 1], F32, name="lmb")
            nc.sync.dma_start(lmb[:], d_lmb[:])
            lnfw = cpool.tile([128, KT], F32, name="lnfw")
            nc.sync.dma_start(lnfw[:], d_lnfw[:])
            lnfb = cpool.tile([128, KT], F32, name="lnfb")
            nc.sync.dma_start(lnfb[:], d_lnfb[:])

            ones_col16 = cpool.tile([128, 1], F16, name="ones_col16")
            nc.scalar.activation(ones_col16[:], ones_col[:], AF.Identity)

            dar_i = dpool.tile([1, 8], F32, name="dar_i")
            nc.sync.dma_start(dar_i[:], d_dar[:])
            dar_o = dpool.tile([1, 8], F32, name="dar_o", addr_space="Shared")
            nc.gpsimd.collective_compute(
                "AllReduce", ALU.add,
                replica_groups=[list(range(N_CORES))],
                ins=[dar_i[:]], outs=[dar_o[:]])

            # AG bounce tensors (per-head o, fp16) and AR tensors (ffn, fp16)
            ag_o_i = [[dpool.tile([H, NHLF], F16, name=f"agi{l}_{h}")
                       for h in range(2)] for l in range(L)]
            ag_o_o = [[dpool.tile([H * N_CORES, NHLF], F16, name=f"ago{l}_{h}",
                                  addr_space="Shared")
                       for h in range(2)] for l in range(L)]
            ar_ffn_i = [[dpool.tile([D, NHLF], F16, name=f"afi{l}_{h}")
                         for h in range(2)] for l in range(L)]
            ar_ffn_o = [[dpool.tile([D, NHLF], F16, name=f"afo{l}_{h}",
                                    addr_space="Shared")
                         for h in range(2)] for l in range(L)]

            # ---- x state: 6 tiles [128, N] fp32 + fp16 shadow ----
            x_sb = xpool.tile([128, KT * N], F32, name="x_sb")
            x16 = xpool.tile([128, KT * N], F16, name="x16")

            def xs(k, hs=slice(0, N)):
                return x_sb[:, k * N + hs.start:k * N + hs.stop]

            def xs16(k, hs=slice(0, N)):
                return x16[:, k * N + hs.start:k * N + hs.stop]

            def hslice(h):
                return slice(h * NHLF, (h + 1) * NHLF)

            # ---- embedding (full batch) ----
            idx_i = sm.tile([1, N], I32, name="idx_i", bufs=1)
            nc.sync.dma_start(idx_i[:], d_idx[:])
            idx_f = sm.tile([1, N], F32, name="idx_f", bufs=1)
            nc.vector.tensor_copy(idx_f[:], idx_i[:])
            onehot = wk.tile([V, N], F32, name="onehot", bufs=1)
            for h in range(2):
                hs = hslice(h)
                idxbc = ps_att.tile([V, NHLF], F32, tag="a", name="idxbc")
                nc.tensor.matmul(idxbc[:], ones_row[:, :V], idx_f[:, hs],
                                 start=True, stop=True)
                nc.vector.tensor_scalar(onehot[:, hs], idxbc[:], iota99[:],
                                        None, op0=ALU.is_equal)
            for k in range(KT):
                posk = sm.tile([128, N], F32, tag="posk", bufs=2)
                nc.sync.dma_start(posk[:], d_posT[k * 128:(k + 1) * 128, :])
                for h in range(2):
                    hs = hslice(h)
                    e_ps = ps_acc.tile([128, NHLF], F32, tag="acc")
                    nc.tensor.matmul(e_ps[:], tok[:, k * 128:(k + 1) * 128],
                                     onehot[:, hs], start=True, stop=True)
                    nc.vector.tensor_add(xs(k, hs), e_ps[:], posk[:, hs])
                    nc.vector.tensor_copy(xs16(k, hs), xs(k, hs))

            def layernorm_emit(q, h, get_t, w_ap, b_ap, make16, uniq,
                               tmp=None):
                """Append LN chunks to q: t chunks, batched sums, finisher,
                6 norm chunks. If tmp given, t is already materialized."""
                hs = hslice(h)
                if tmp is None:
                    tmp = wk.tile([128, KT * NHLF], F32, name=f"ln_t{uniq}",
                                  bufs=1)

                    def mk_t(k):
                        def go():
                            get_t(k, tmp[:, k * NHLF:(k + 1) * NHLF])
                        return go
                    for k in range(KT):
                        q.append(mk_t(k))
                st = {}

                def sums():
                    st["s"] = ps_ln.tile([1, NHLF], F32, tag="lns",
                                         name="ln_s")[:]
                    st["q"] = ps_ln.tile([1, NHLF], F32, tag="lns",
                                         name="ln_q")[:]
                    for k in range(KT):
                        tk = tmp[:, k * NHLF:(k + 1) * NHLF]
                        sq = sm.tile([128, NHLF], F32, tag="lnsq", bufs=3)
                        nc.scalar.activation(sq[:], tk, AF.Square)
                        nc.tensor.matmul(st["s"], ones_col[:], tk,
                                         start=(k == 0), stop=(k == KT - 1))
                        nc.tensor.matmul(st["q"], ones_col[:], sq[:],
                                         start=(k == 0), stop=(k == KT - 1))

                q.append(sums)

                def finishA():
                    murd = sm.tile([1, 2 * NHLF], F32, tag="ln1", bufs=2)
                    mu = murd[:, :NHLF]
                    rstd = murd[:, NHLF:]
                    nc.vector.tensor_scalar_mul(mu, st["s"], 1.0 / D)
                    mu2 = sm.tile([1, NHLF], F32, tag="ln2", bufs=2)
                    nc.vector.tensor_mul(mu2[:], mu, mu)
                    var = sm.tile([1, NHLF], F32, tag="ln3", bufs=2)
                    nc.vector.scalar_tensor_tensor(var[:], st["q"], 1.0 / D,
                                                   mu2[:], op0=ALU.mult,
                                                   op1=ALU.subtract)
                    sd = sm.tile([1, NHLF], F32, tag="ln4", bufs=2)
                    nc.scalar.activation(sd[:], var[:], AF.Sqrt, bias=epsb[:])
                    nc.vector.reciprocal(rstd, sd[:])
                    st["murd"] = murd

                def finishB():
                    bc = ps_bc.tile([128, 2 * NHLF], F32, tag="bc")
                    nc.tensor.matmul(bc[:], ones_row[:], st["murd"][:],
                                     start=True, stop=True)
                    st["bc"] = bc

                def mk_norm(k):
                    def go():
                        mu_bc = st["bc"][:, :NHLF]
                        rstd_bc = st["bc"][:, NHLF:]
                        tk = tmp[:, k * NHLF:(k + 1) * NHLF]
                        u = sm.tile([128, NHLF], F32, tag="lnu", bufs=2)
                        nc.vector.scalar_tensor_tensor(u[:], tk, 1.0, mu_bc,
                                                       op0=ALU.mult,
                                                       op1=ALU.subtract)
                        nc.vector.tensor_mul(u[:], u[:], rstd_bc)
                        nc.vector.tensor_scalar(xs(k, hs), u[:],
                                                w_ap[:, k:k + 1],
                                                b_ap[:, k:k + 1],
                                                op0=ALU.mult, op1=ALU.add)
                        if make16:
                            nc.vector.tensor_copy(xs16(k, hs),
                                                  xs(k, hs))
                    return go

                q.append(finishA)
                q.append(finishB)
                for k in range(KT):
                    q.append(mk_norm(k))

            def load_attn_weights(l):
                aw = {}
                aw["wq"] = awpool.tile([128, KT * H], F16, name="wq", tag="wq")
                aw["wk"] = awpool.tile([128, KT * H], F16, name="wk", tag="wk")
                aw["wv"] = awpool.tile([128, KT * H], F16, name="wv", tag="wv")
                nc.sync.dma_start(aw["wq"][:].rearrange("p (k h) -> p k h",
                                                        k=KT), d_wqT[l])
                nc.sync.dma_start(aw["wk"][:].rearrange("p (k h) -> p k h",
                                                        k=KT), d_wkT[l])
                nc.sync.dma_start(aw["wv"][:].rearrange("p (k h) -> p k h",
                                                        k=KT), d_wvT[l])
                aw["wp"] = awpool.tile([128, KT * D], F16, name="wp", tag="wp")
                nc.sync.dma_start(aw["wp"][:].rearrange("p (k d) -> p k d",
                                                        k=KT), d_wpT[l])
                aw["bpj"] = awpool.tile([128, KT], F32, name="bpj", tag="bpj")
                nc.sync.dma_start(aw["bpj"][:], d_bproj[l])
                aw["l1w"] = awpool.tile([128, KT], F32, name="l1w", tag="l1w")
                nc.sync.dma_start(aw["l1w"][:], d_ln1w[l])
                aw["l1b"] = awpool.tile([128, KT], F32, name="l1b", tag="l1b")
                nc.sync.dma_start(aw["l1b"][:], d_ln1b[l])
                return aw

            def load_moe_weights(l):
                mw = {}
                mw["gw"] = awpool.tile([128, KT * E], F32, name="gw", tag="gw")
                nc.sync.dma_start(mw["gw"][:].rearrange("p (k e) -> p k e",
                                                        k=KT), d_gwT[l])
                mw["gb"] = awpool.tile([1, E], F32, name="gb", tag="gb")
                nc.sync.dma_start(mw["gb"][:], d_gb[l])
                mw["l2w"] = awpool.tile([128, KT], F32, name="l2w", tag="l2w")
                nc.sync.dma_start(mw["l2w"][:], d_ln2w[l])
                mw["l2b"] = awpool.tile([128, KT], F32, name="l2b", tag="l2b")
                nc.sync.dma_start(mw["l2b"][:], d_ln2b[l])
                mw["b1t"] = awpool.tile([128, MT], F32, name="b1t", tag="b1t")
                nc.sync.dma_start(mw["b1t"][:], d_b1[l])
                mw["b2own"] = awpool.tile([128, KT], F32, name="b2own",
                                          tag="b2t")
                nc.sync.dma_start(mw["b2own"][:], d_b2c[l])
                return mw

            def attn_phase(l, h, aw):
                """This core's head for half h -> o fp16 -> AllGather."""
                hs = hslice(h)
                q_ps = ps_acc.tile([H, NHLF], F32, tag="acc")
                k_ps = ps_acc.tile([H, NHLF], F32, tag="acc")
                v_ps = ps_acc.tile([H, NHLF], F32, tag="acc")
                for k in range(KT):
                    nc.tensor.matmul(q_ps[:], aw["wq"][:, k * H:(k + 1) * H],
                                     xs16(k, hs), start=(k == 0),
                                     stop=(k == KT - 1))
                for k in range(KT):
                    nc.tensor.matmul(k_ps[:], aw["wk"][:, k * H:(k + 1) * H],
                                     xs16(k, hs), start=(k == 0),
                                     stop=(k == KT - 1))
                for k in range(KT):
                    nc.tensor.matmul(v_ps[:], aw["wv"][:, k * H:(k + 1) * H],
                                     xs16(k, hs), start=(k == 0),
                                     stop=(k == KT - 1))
                qT = wk.tile([H, NHLF], F32, name=f"qT{h}", bufs=1)
                kT_ = wk.tile([H, NHLF], F32, name=f"kT{h}", bufs=1)
                vT = wk.tile([H, NHLF], F32, name=f"vT{h}", bufs=1)
                nc.vector.tensor_copy(qT[:], q_ps[:])
                nc.vector.tensor_copy(kT_[:], k_ps[:])
                nc.vector.tensor_copy(vT[:], v_ps[:])

                oT = wk.tile([H, NHLF], F16, name=f"oT{h}", bufs=1)
                w4 = ps_att.tile([64, NHLF], F32, tag="a", name="w4")
                for b in range(4):
                    ts_ = slice(b * 64, (b + 1) * 64)
                    nc.tensor.matmul(w4[:, ts_], qT[:, ts_], kT_[:, ts_],
                                     start=True, stop=True)
                s4 = sm.tile([64, NHLF], F32, tag="att_s", bufs=1)
                nc.vector.scalar_tensor_tensor(s4[:], w4[:], SCALE,
                                               mask4[:], op0=ALU.mult,
                                               op1=ALU.add)
                ssum4 = sm.tile([64, 4], F32, tag="att_su", bufs=1)
                rs4 = sm.tile([64, 4], F32, tag="att_r", bufs=1)
                att4 = sm.tile([64, NHLF], F32, tag="att_a", bufs=1)
                for b in range(4):
                    ts_ = slice(b * 64, (b + 1) * 64)
                    mx = sm.tile([64, 1], F32, tag="att_m", bufs=2)
                    nc.vector.reduce_max(mx[:], s4[:, ts_], axis=AX.X,
                                         negate=True)
                    nc.scalar.activation(att4[:, ts_], s4[:, ts_], AF.Exp,
                                         bias=mx[:],
                                         accum_out=ssum4[:, b:b + 1])
                nc.vector.reciprocal(rs4[:], ssum4[:])
                for b in range(4):
                    ts_ = slice(b * 64, (b + 1) * 64)
                    nc.vector.tensor_scalar_mul(att4[:, ts_], att4[:, ts_],
                                                rs4[:, b:b + 1])
                at4 = ps_att.tile([64, NHLF], F32, tag="a", name="at4")
                for b in range(4):
                    ts_ = slice(b * 64, (b + 1) * 64)
                    nc.tensor.transpose(at4[:, ts_], att4[:, ts_],
                                        ident[:64, :64])
                attT4 = sm.tile([64, NHLF], F32, tag="att_t", bufs=1)
                nc.vector.tensor_copy(attT4[:], at4[:])
                vt4 = ps_att.tile([64, 4 * H], F32, tag="a", name="vt4")
                for b in range(4):
                    nc.tensor.transpose(vt4[:, b * H:(b + 1) * H],
                                        vT[:, b * 64:(b + 1) * 64],
                                        ident[:H, :H])
                vtb4 = sm.tile([64, 4 * H], F32, tag="att_v", bufs=1)
                nc.vector.tensor_copy(vtb4[:], vt4[:])
                o4 = ps_att.tile([H, NHLF], F32, tag="a", name="o4")
                for b in range(4):
                    ts_ = slice(b * 64, (b + 1) * 64)
                    nc.tensor.matmul(o4[:, ts_],
                                     vtb4[:, b * H:(b + 1) * H],
                                     attT4[:, ts_], start=True, stop=True)
                nc.vector.tensor_copy(oT[:], o4[:])

                nc.sync.dma_start(ag_o_i[l][h][:], oT[:])
                nc.gpsimd.collective_compute(
                    "AllGather", ALU.bypass,
                    replica_groups=[list(range(N_CORES))],
                    ins=[ag_o_i[l][h][:]], outs=[ag_o_o[l][h][:]])

            def proj_ln1_gate_emit(q, l, h, aw, mw):
                """Append chunks: o16 load, 6 proj, 13 LN, 5 gate. Returns
                (combT, cbc, comb16) tiles filled when chunks run."""
                hs = hslice(h)
                o16 = wk.tile([128, KT * NHLF], F16, name=f"o16{h}", bufs=1)
                t1 = wk.tile([128, KT * NHLF], F32, name=f"t1_{h}", bufs=1)
                combT = sm.tile([E, NHLF], F32, name=f"combT{h}", bufs=1)
                cbc = wk.tile([128, NHLF], F32, name=f"cbc{h}", bufs=1)

                def load_o():
                    for k in range(KT):
                        nc.sync.dma_start(o16[:, k * NHLF:(k + 1) * NHLF],
                                          ag_o_o[l][h][k * 128:(k + 1) * 128, :])
                q.append(load_o)

                def mk_proj(m):
                    def go():
                        y_ps = ps_acc.tile([128, NHLF], F32, tag="acc")
                        for k in range(KT):
                            nc.tensor.matmul(
                                y_ps[:],
                                aw["wp"][:, k * D + m * 128:
                                         k * D + (m + 1) * 128],
                                o16[:, k * NHLF:(k + 1) * NHLF],
                                start=(k == 0), stop=(k == KT - 1))
                        # t = (y + bproj) + x, read straight from PSUM
                        nc.vector.scalar_tensor_tensor(
                            t1[:, m * NHLF:(m + 1) * NHLF], y_ps[:],
                            aw["bpj"][:, m:m + 1], xs(m, hs),
                            op0=ALU.add, op1=ALU.add)
                    return go
                for m in range(KT):
                    q.append(mk_proj(m))

                layernorm_emit(q, h, None, aw["l1w"], aw["l1b"], True,
                               f"1_{h}", tmp=t1)

                g_pss = []

                def mk_glog(tt):
                    def go():
                        g_ps = ps_att.tile([128, E], F32, tag="a", name="g_ps")
                        for k in range(KT):
                            nc.tensor.matmul(
                                g_ps[:],
                                x_sb[:, k * N + hs.start + tt * 128:
                                     k * N + hs.start + (tt + 1) * 128],
                                mw["gw"][:, k * E:(k + 1) * E],
                                start=(k == 0), stop=False)
                        nc.tensor.matmul(g_ps[:], ones_row[:], mw["gb"][:],
                                         start=False, stop=True)
                        g_pss.append(g_ps)
                    return go

                cws = []

                def mk_smax(tt):
                    def go():
                        g_ps = g_pss[tt]
                        mx = sm.tile([128, 1], F32, tag="g_m")
                        nc.vector.reduce_max(mx[:], g_ps[:], axis=AX.X,
                                             negate=True)
                        pr = sm.tile([128, E], F32, tag="g_p")
                        ssum = sm.tile([128, 1], F32, tag="g_s")
                        nc.scalar.activation(pr[:], g_ps[:], AF.Exp, bias=mx[:],
                                             accum_out=ssum[:])
                        rs = sm.tile([128, 1], F32, tag="g_r")
                        nc.vector.reciprocal(rs[:], ssum[:])
                        nc.vector.tensor_scalar_mul(pr[:], pr[:], rs[:])
                        top8 = sm.tile([128, 8], F32, tag="g_t8")
                        nc.vector.max(out=top8[:], in_=pr[:])
                        msk = sm.tile([128, E], F32, tag="g_msk")
                        nc.vector.tensor_scalar(msk[:], pr[:], top8[:, 1:2],
                                                None, op0=ALU.is_ge)
                        cw = sm.tile([128, E], F32, tag=f"g_cw{tt}", bufs=1)
                        nc.vector.tensor_mul(cw[:], pr[:], msk[:])
                        den = sm.tile([128, 1], F32, tag="g_den")
                        nc.vector.tensor_add(den[:], top8[:, 0:1], top8[:, 1:2])
                        dr = sm.tile([128, 1], F32, tag="g_dr")
                        nc.vector.reciprocal(dr[:], den[:])
                        nc.vector.tensor_scalar_mul(cw[:], cw[:], dr[:])
                        cws.append(cw)
                    return go

                def crow_cbc():
                    for tt in range(2):
                        ct_ps = ps_att.tile([E, 128], F32, tag="a", name="ct_ps")
                        nc.tensor.transpose(ct_ps[:], cws[tt][:], ident[:])
                        nc.vector.tensor_copy(
                            combT[:, tt * 128:(tt + 1) * 128], ct_ps[:])
                    crow_ps = ps_att.tile([1, NHLF], F32, tag="a", name="crow_ps")
                    nc.tensor.matmul(crow_ps[:], combsel[:], combT[:],
                                     start=True, stop=True)
                    crow = sm.tile([1, NHLF], F32, tag=f"crow{h}", bufs=1)
                    nc.vector.tensor_copy(crow[:], crow_ps[:])
                    cbc_ps = ps_bc.tile([128, NHLF], F32, tag="bc")
                    nc.tensor.matmul(cbc_ps[:], ones_row[:], crow[:],
                                     start=True, stop=True)
                    nc.vector.tensor_copy(cbc[:], cbc_ps[:])

                q.append(mk_glog(0))
                q.append(mk_glog(1))
                q.append(mk_smax(0))
                q.append(mk_smax(1))
                q.append(crow_cbc)
                return combT, cbc

            def ffn_phase(l, h, mw, cbc, q, w1_slots=True):
                """Dense expert FFN for half h (fp16), -> AllReduce.

                Pops one pending chunk from q after each w1 m-tile (if
                w1_slots) and two after each w2 m-tile."""
                hs = hslice(h)
                hT = hpool.tile([128, MT * NHLF], F16, tag="hT")

                def fill(n):
                    for _ in range(n):
                        if q:
                            f = q.pop(0)
                            if f is not None:
                                f()

                for m in range(MT):
                    w1m = w1pool.tile([128, KT * 128], F16, tag="w1")
                    nc.sync.dma_start(
                        w1m[:].rearrange("p (k f) -> p k f", k=KT),
                        d_w1h[l, m])
                    h_ps = ps_acc.tile([128, NHLF], F32, tag="acc")
                    for k in range(KT):
                        nc.tensor.matmul(
                            h_ps[:], w1m[:, k * 128:(k + 1) * 128],
                            xs16(k, hs),
                            start=(k == 0), stop=(k == KT - 1))
                    nc.scalar.activation(
                        hT[:, m * NHLF:(m + 1) * NHLF],
                        h_ps[:], AF.Gelu, bias=mw["b1t"][:, m:m + 1])
                    if w1_slots:
                        fill(1)
                for m in range(KT):
                    w2m = w2pool.tile([128, MT * 128], F16, tag="w2")
                    nc.sync.dma_start(
                        w2m[:].rearrange("p (j f) -> p j f", j=MT),
                        d_w2h[l, m])
                    ye_ps = ps_acc.tile([128, NHLF], F32, tag="acc")
                    for j in range(MT):
                        nc.tensor.matmul(
                            ye_ps[:], w2m[:, j * 128:(j + 1) * 128],
                            hT[:, j * NHLF:(j + 1) * NHLF],
                            start=(j == 0), stop=(j == MT - 1))
                    ysc = sm.tile([128, NHLF], F16, tag="ycp", bufs=2)
                    nc.vector.scalar_tensor_tensor(
                        ysc[:], ye_ps[:], mw["b2own"][:, m:m + 1], cbc[:],
                        op0=ALU.add, op1=ALU.mult)
                    nc.sync.dma_start(
                        ar_ffn_i[l][h][m * 128:(m + 1) * 128, :], ysc[:])
                    fill(3)
                nc.gpsimd.collective_compute(
                    "AllReduce", ALU.add,
                    replica_groups=[list(range(N_CORES))],
                    ins=[ar_ffn_i[l][h][:]], outs=[ar_ffn_o[l][h][:]])

            def ln2_emit(q, l, h, mw, combT, make16):
                """Append chunks: ym16 load + LN2 (residual + b2@comb)."""
                ym16 = wk.tile([128, KT * NHLF], F16, name=f"ym16{h}", bufs=1)
                hs = hslice(h)

                def load_ym():
                    for k in range(KT):
                        nc.sync.dma_start(ym16[:, k * NHLF:(k + 1) * NHLF],
                                          ar_ffn_o[l][h][k * 128:(k + 1) * 128,
                                                         :])
                q.append(load_ym)

                def get_t2(k, dst):
                    nc.vector.tensor_add(dst, ym16[:, k * NHLF:(k + 1) * NHLF],
                                         xs(k, hs))

                layernorm_emit(q, h, get_t2, mw["l2w"], mw["l2b"], make16,
                               f"2_{h}")

            def final_phase(h):
                """Final layernorm + lm head for half h."""
                hs = hslice(h)

                def get_tf(k, dst):
                    nc.vector.tensor_copy(dst, xs(k, hs))

                qf = []
                layernorm_emit(qf, h, get_tf, lnfw, lnfb, True, f"f_{h}")
                for f in qf:
                    f()
                lg_ps = ps_acc.tile([V, NHLF], F32, tag="acc")
                for k in range(KT):
                    nc.tensor.matmul(
                        lg_ps[:], lmT[:, k * V:(k + 1) * V],
                        xs16(k, hs),
                        start=(k == 0), stop=(k == KT - 1))
                lg = sm.tile([V, NHLF], F32, tag=f"lgout{h}", bufs=1)
                nc.scalar.activation(lg[:], lg_ps[:], AF.Identity, bias=lmb[:])
                nc.sync.dma_start(d_out[:, hs], lg[:])

            # ---- software-pipelined layer loop ----
            aw = load_attn_weights(0)
            attn_phase(0, 0, aw)
            attn_phase(0, 1, aw)
            for l in range(L):
                mw = load_moe_weights(l)
                q0 = []
                combT0, cbc0 = proj_ln1_gate_emit(q0, l, 0, aw, mw)
                for f in q0:
                    f()
                if l < L - 1:
                    aw2 = load_attn_weights(l + 1)
                q1 = []
                combT1, cbc1 = proj_ln1_gate_emit(q1, l, 1, aw, mw)
                ffn_phase(l, 0, mw, cbc0, q1)
                for f in q1:
                    f()
                # pad so ln2 chunks pop only once AR(l,0) has landed
                # (~20us after its trigger at ffn(l,0) end); the ym16 DMA
                # then never stalls the sync queue ahead of w1 prefetch.
                q2 = [None] * 20
                ln2_emit(q2, l, 0, mw, combT0,
                         make16=(l < L - 1))
                ffn_phase(l, 1, mw, cbc1, q2, w1_slots=True)
                for f in q2:
                    if f is not None:
                        f()
                if l < L - 1:
                    attn_phase(l + 1, 0, aw2)
                    q3 = []
                    ln2_emit(q3, l, 1, mw, combT1, make16=True)
                    for f in q3:
                        f()
                    attn_phase(l + 1, 1, aw2)
                    aw = aw2
                else:
                    final_phase(0)
                    q3 = []
                    ln2_emit(q3, l, 1, mw, combT1, make16=False)
                    for f in q3:
                        f()
                    final_phase(1)

    return nc


def _prep(inputs):
    """Build per-core input maps from the full input dict."""
    f = lambda a: np.ascontiguousarray(np.asarray(a), dtype=np.float32)
    h = lambda a: np.ascontiguousarray(np.asarray(a), dtype=np.float16)
    idx = np.asarray(inputs["idx"]).reshape(1, N)
    wq, wkk, wv = f(inputs["wq"]), f(inputs["wk"]), f(inputs["wv"])
    wproj, bproj = f(inputs["wproj"]), f(inputs["bproj"])
    gate_w, gate_b = f(inputs["gate_w"]), f(inputs["gate_b"])
    w1, b1 = f(inputs["w1"]), f(inputs["b1"])
    w2, b2 = f(inputs["w2"]), f(inputs["b2"])

    base = {
        "idx": np.ascontiguousarray(idx.astype(np.int32)),
        "iota99": np.arange(V, dtype=np.float32).reshape(V, 1),
        "ident128": np.eye(128, dtype=np.float32),
        "maskb": np.where(np.tril(np.ones((64, 64), bool)), 0.0,
                          -1e30).astype(np.float32),
        "ones_col": np.ones((128, 1), np.float32),
        "ones_row": np.ones((1, 128), np.float32),
        "tok_emb": f(inputs["tok_emb"]),
        "posT": np.ascontiguousarray(
            np.tile(f(inputs["pos_emb"]).T, (1, B))),
        # full wproj^T for every core: [in 768 -> KT,128][out 768]
        "wpT": np.ascontiguousarray(
            wproj.transpose(0, 2, 1).reshape(L, KT, 128, D)
            .transpose(0, 2, 1, 3)).astype(np.float16),
        "gwT": np.ascontiguousarray(
            gate_w.transpose(0, 2, 1).reshape(L, KT, 128, E)
            .transpose(0, 2, 1, 3)),
        "gb": gate_b.reshape(L, 1, E),
        
        "ln1w": np.ascontiguousarray(f(inputs["ln1_w"]).reshape(L, KT, 128).transpose(0, 2, 1)),
        "ln1b": np.ascontiguousarray(f(inputs["ln1_b"]).reshape(L, KT, 128).transpose(0, 2, 1)),
        "ln2w": np.ascontiguousarray(f(inputs["ln2_w"]).reshape(L, KT, 128).transpose(0, 2, 1)),
        "ln2b": np.ascontiguousarray(f(inputs["ln2_b"]).reshape(L, KT, 128).transpose(0, 2, 1)),
        "lnfw": np.ascontiguousarray(f(inputs["lnf_w"]).reshape(KT, 128).T),
        "lnfb": np.ascontiguousarray(f(inputs["lnf_b"]).reshape(KT, 128).T),
        "lmT": np.ascontiguousarray(
            f(inputs["lm_w"]).T.reshape(KT, 128, V)).astype(np.float16),
        "dar": np.zeros((1, 8), np.float32),
        "epsb": np.full((1, 1), EPS, np.float32),
        "lmb": f(inputs["lm_b"]).reshape(V, 1),
        "bproj": np.ascontiguousarray(bproj.reshape(L, KT, 128).transpose(0, 2, 1)),
    }
    in_maps = []
    for c in range(N_CORES):
        m = dict(base)
        m["wqT"] = np.ascontiguousarray(
            wq[:, c].transpose(0, 2, 1).reshape(L, KT, 128, H)
            .transpose(0, 2, 1, 3)).astype(np.float16)
        m["wkT"] = np.ascontiguousarray(
            wkk[:, c].transpose(0, 2, 1).reshape(L, KT, 128, H)
            .transpose(0, 2, 1, 3)).astype(np.float16)
        m["wvT"] = np.ascontiguousarray(
            wv[:, c].transpose(0, 2, 1).reshape(L, KT, 128, H)
            .transpose(0, 2, 1, 3)).astype(np.float16)
        w1tc = w1[:, c].transpose(0, 2, 1)  # [L, 768, 3072]
        m["w1h"] = np.ascontiguousarray(
            w1tc.reshape(L, KT, 128, MT, 128).transpose(0, 3, 2, 1, 4)).astype(
                np.float16)
        m["b1"] = np.ascontiguousarray(b1[:, c].reshape(L, MT, 128).transpose(0, 2, 1))
        w2tc = w2[:, c].transpose(0, 2, 1)  # [L, 3072, 768]
        m["w2h"] = np.ascontiguousarray(
            w2tc.reshape(L, MT, 128, KT, 128).transpose(0, 3, 2, 1, 4)).astype(
                np.float16)
        m["b2own"] = np.ascontiguousarray(
            b2[:, c].reshape(L, KT, 128).transpose(0, 2, 1))
        sel = np.zeros((E, 1), np.float32)
        sel[c, 0] = 1.0
        m["combsel"] = sel
        in_maps.append(m)
    return in_maps


def kernel(**inputs) -> np.ndarray:
    if "nc" not in _CACHED:
        _CACHED["nc"] = build()
    nc = _CACHED["nc"]
    in_maps = _prep(inputs)
    res = run_bass_kernel_spmd(nc, in_maps, list(range(N_CORES)))
    lt = res.results[0]["logitsT"]  # [V, N]
    return np.ascontiguousarray(lt.T.reshape(B, T, V).astype(np.float32))


if __name__ == "__main__":
    import jax

    jax.config.update("jax_platforms", "cpu")
    import reference as ref

    inp = ref.setup_inputs()
    want = np.asarray(ref.reference(**inp))
    import jax as _j
    _j.config.update("jax_platforms", "axon")
    got = kernel(**{k: np.asarray(v) for k, v in inp.items()})
    err = np.abs(got - want).max()
    rel = err / np.abs(want).max()
    l2 = np.linalg.norm(got - want) / np.linalg.norm(want)
    print(f"absmax {err:.3e}  absmax-rel {rel:.3e}  l2-rel {l2:.3e}")
